# revision 28
# baseline (speedup 1.0000x reference)
"""AlphaFold-style NoGatingAttention on 8 Trainium2 NeuronCores.

Problem (hardcoded): B=128, Q=K=384, A=M=256, H=8, KD=VD=32, OUT=256, fp32 I/O.

Strategy: data-parallel over batch (16 per core). Per batch, on-device:
  qT = Wq^T @ q_data^T            [hc, q]   (scale folded into Wq on host)
  kT = Wk^T @ m_data^T            [hc, k]
  v  = m_data^T.T @ Wv            [k, hc]   (natural layout)
  logits^T[k,q] per head = nb^T (PE-injected) + kT_h^T-slices @ qT_h
  E = exp(logits^T)                          (ScalarE, fp16 out)
  wa[q, (h,33)] = E_slice^T @ v_ext          (v_ext has EB=exp(bias) folded in;
                                              col 32 of each head = softmax denom)
  wa_n = wa * recip(denom)  -> PE transpose -> [hc, q]
  out[q, o] = wa_n^T-chunks @ Wo             (+ output_b added on host)

All matmuls fp16 (1 cycle/row on PE); PSUM accumulation fp32.
"""

import numpy as np

import concourse.bass as bass
import concourse.mybir as mybir
import concourse.tile as tile
from concourse import bacc
from concourse.bass_utils import run_bass_kernel_spmd
from concourse.masks import make_identity

B, Q, KL, A_DIM, H, KD, VD, OUT = 128, 384, 384, 256, 8, 32, 32, 256
NCORES = 8
BPC = B // NCORES  # 16 batches per core
HC = H * KD  # 256
F16 = mybir.dt.float16
F32 = mybir.dt.float32

# (t, g) head-pair groups whose nonbatched-bias add runs as a post-exp
# multiply by exp(nb) on DVE / GpSimd instead of a PE PSUM-inject
# (PE/DVE/GpSimd balance).
OFFLOAD_DVE = {(0, 3), (1, 3), (2, 3), (0, 2), (1, 2)}
OFFLOAD_GPS = {(2, 2), (0, 1), (1, 1)}
OFFLOAD = OFFLOAD_DVE | OFFLOAD_GPS
OFF_HEADS = sorted({g * 2 + j for (_, g) in OFFLOAD for j in range(2)})

_CACHE = {}


def _build_program():
    """Build the per-core Bass/Tile program (identical on all 8 cores)."""
    nc = bacc.Bacc("TRN2", target_bir_lowering=False, debug=False)

    # --- per-core DRAM I/O ---
    qT_d = nc.dram_tensor("qT", [BPC, A_DIM, Q], F16, kind="ExternalInput")
    mT_d = nc.dram_tensor("mT", [BPC, A_DIM, KL], F16, kind="ExternalInput")
    eb_d = nc.dram_tensor("EB", [BPC, KL], F32, kind="ExternalInput")
    nb_d = nc.dram_tensor("nbT", [128, 3, H, Q], F16, kind="ExternalInput")
    enb_d = nc.dram_tensor(
        "ENB", [128, 3, len(OFF_HEADS), Q], F16, kind="ExternalInput"
    )
    wq_d = nc.dram_tensor("Wq", [2, 128, HC], F16, kind="ExternalInput")
    wk_d = nc.dram_tensor("Wk", [2, 128, HC], F16, kind="ExternalInput")
    wv_d = nc.dram_tensor("Wv", [2, 128, HC], F16, kind="ExternalInput")
    wo_d = nc.dram_tensor("Wo", [2, 128, OUT], F16, kind="ExternalInput")
    out_d = nc.dram_tensor("out", [BPC, 3, 128, OUT], F32, kind="ExternalOutput")

    MUL = mybir.AluOpType.mult

    with tile.TileContext(nc) as tc:
        with (
            tc.tile_pool(name="const", bufs=1) as constp,
            tc.tile_pool(name="io", bufs=3) as iop,
            tc.tile_pool(name="work", bufs=2) as workp,
            tc.tile_pool(name="epool", bufs=2) as ep,
            tc.tile_pool(name="psL", bufs=2, space="PSUM") as psL,
            tc.tile_pool(name="psP", bufs=2, space="PSUM") as psP,
            tc.tile_pool(name="psW", bufs=2, space="PSUM") as psW,
        ):
            # --- constants (loaded once) ---
            ident = constp.tile([128, 128], F16)
            make_identity(nc, ident)

            # constants go on the gpsimd (SWDGE) queue so the first batch's
            # input DMAs on the sync queue aren't stuck behind them
            nb_sb = constp.tile([128, 3, H, Q], F16)
            nc.gpsimd.dma_start(out=nb_sb, in_=nb_d[:])
            enb_sb = constp.tile([128, 3, len(OFF_HEADS), Q], F16)
            nc.gpsimd.dma_start(out=enb_sb, in_=enb_d[:])
            off_idx = {h: i for i, h in enumerate(OFF_HEADS)}

            w_sb = {}
            for name, d in (("q", wq_d), ("k", wk_d), ("v", wv_d), ("o", wo_d)):
                w = constp.tile([128, 2, HC], F16, tag=f"w{name}")
                nc.gpsimd.dma_start(out=w, in_=d.rearrange("c p n -> p c n"))
                w_sb[name] = w

            for b in range(BPC):
                # --- input DMAs ---
                qTt = iop.tile([128, 2, Q], F16, tag="qT")
                nc.sync.dma_start(
                    out=qTt, in_=qT_d[b].rearrange("(c p) q -> p c q", p=128)
                )
                mTt = iop.tile([128, 2, KL], F16, tag="mT")
                nc.sync.dma_start(
                    out=mTt, in_=mT_d[b].rearrange("(c p) q -> p c q", p=128)
                )
                ebt = iop.tile([128, 3], F32, tag="eb")
                nc.sync.dma_start(
                    out=ebt, in_=eb_d[b].rearrange("(t p) -> p t", p=128)
                )

                # --- projections: qT [hc, q], kT [hc, k] ---
                proj = {}
                for name, src in (("q", qTt), ("k", mTt)):
                    dst = workp.tile([128, 2, Q], F16, tag=f"p{name}")
                    for t in range(2):
                        ps = psP.tile([128, Q], F32, tag="ps")
                        for c in range(2):
                            nc.tensor.matmul(
                                ps,
                                w_sb[name][:, c, t * 128 : (t + 1) * 128],
                                src[:, c, :],
                                start=(c == 0),
                                stop=(c == 1),
                            )
                        nc.vector.tensor_copy(dst[:, t, :], ps)
                    proj[name] = dst

                # --- v natural [k, hc] scaled by EB, with EB ones-columns ---
                # layout: [128, kt, h*33 + c]; col 33h+32 = EB (denominator trick)
                v_sb = workp.tile([128, 3, H * 33], F16, tag="v")
                for t in range(3):
                    ps = psP.tile([128, HC], F32, tag="ps")
                    for c in range(2):
                        nc.tensor.matmul(
                            ps,
                            mTt[:, c, t * 128 : (t + 1) * 128],
                            w_sb["v"][:, c, :],
                            start=(c == 0),
                            stop=(c == 1),
                        )
                    nc.vector.tensor_scalar(
                        out=v_sb[:, t].rearrange("p (h x) -> p h x", x=33)[:, :, 0:32],
                        in0=ps.rearrange("p (h x) -> p h x", x=32),
                        scalar1=ebt[:, t : t + 1],
                        scalar2=None,
                        op0=MUL,
                    )
                    # ones-columns = EB broadcast into col 32 of each head block
                    nc.vector.tensor_copy(
                        v_sb[:, t].rearrange("p (h x) -> p h x", x=33)[:, :, 32],
                        ebt[:, t : t + 1].broadcast_to((128, H)),
                    )

                # --- logits^T + exp, per k-tile and head-pair ---
                E_sb = ep.tile([128, 3, H, Q], F16, tag="E")
                for t in range(3):
                    for g in range(4):  # head pairs
                        psl = psL.tile([128, 2, 512], F32, tag="psl")
                        off = (t, g) in OFFLOAD
                        if not off:
                            # inject nonbatched bias into PSUM
                            for j in range(2):
                                h = g * 2 + j
                                nc.tensor.matmul(
                                    psl[:, j, 0:Q],
                                    ident,
                                    nb_sb[:, t, h, :],
                                    start=True,
                                    stop=False,
                                )
                        # logits^T[k, q] = kT_h-slice.T @ qT_h; the two heads
                        # use distinct PE row-groups -> run concurrently
                        for j in range(2):
                            h = g * 2 + j
                            r, ch = (h % 4) * 32, h // 4
                            nc.tensor.matmul(
                                psl[:, j, 0:Q],
                                proj["k"][r : r + 32, ch, t * 128 : (t + 1) * 128],
                                proj["q"][r : r + 32, ch, :],
                                start=off,
                                stop=True,
                                tile_position=(r, 0),
                            )
                        nc.scalar.activation(
                            out=E_sb[:, t, g * 2 : g * 2 + 2, :],
                            in_=psl[:, :, 0:Q],
                            func=mybir.ActivationFunctionType.Exp,
                        )
                        if off:
                            oi = off_idx[g * 2]
                            eng = (
                                nc.gpsimd
                                if (t, g) in OFFLOAD_GPS
                                else nc.vector
                            )
                            eng.tensor_tensor(
                                out=E_sb[:, t, g * 2 : g * 2 + 2, :],
                                in0=E_sb[:, t, g * 2 : g * 2 + 2, :],
                                in1=enb_sb[:, t, oi : oi + 2, :],
                                op=MUL,
                            )

                # --- weighted average + denominators ---
                waT = workp.tile([128, 2, Q], F16, tag="waT")
                for qb in range(3):
                    psw = psW.tile([128, H * 33], F32, tag="ps")
                    for h in range(8):
                        for t in range(3):
                            nc.tensor.matmul(
                                psw[:, h * 33 : h * 33 + 33],
                                E_sb[:, t, h, qb * 128 : (qb + 1) * 128],
                                v_sb[:, t, h * 33 : h * 33 + 33],
                                start=(t == 0),
                                stop=(t == 2),
                            )
                    rden = workp.tile([128, H], F32, tag="rden")
                    nc.vector.reciprocal(
                        rden, psw.rearrange("p (h x) -> p h x", x=33)[:, :, 32]
                    )
                    # normalize all heads in one op: in1 is a stride-0
                    # broadcast of each head's recip denom across its 32 cols
                    wa_n = workp.tile([128, HC], F16, tag="wan")
                    nc.vector.tensor_tensor(
                        out=wa_n.rearrange("p (h x) -> p h x", x=32),
                        in0=psw.rearrange("p (h x) -> p h x", x=33)[:, :, 0:32],
                        in1=rden.rearrange("p (h o) -> p h o", o=1).broadcast_to(
                            (128, H, 32)
                        ),
                        op=MUL,
                    )
                    # transpose to [hc, q]
                    for c in range(2):
                        pst = psW.tile([128, 128], F16, tag="ps")
                        nc.tensor.transpose(
                            pst, wa_n[:, c * 128 : (c + 1) * 128], ident
                        )
                        nc.vector.tensor_copy(
                            waT[:, c, qb * 128 : (qb + 1) * 128], pst
                        )

                # --- output projection ---
                out_sb = iop.tile([128, 3, OUT], F32, tag="out")
                for qb in range(3):
                    pso = psW.tile([128, OUT], F32, tag="ps")
                    for c in range(2):
                        nc.tensor.matmul(
                            pso,
                            waT[:, c, qb * 128 : (qb + 1) * 128],
                            w_sb["o"][:, c, :],
                            start=(c == 0),
                            stop=(c == 1),
                        )
                    nc.vector.tensor_copy(out_sb[:, qb, :], pso)

                nc.sync.dma_start(
                    out=out_d[b].rearrange("t p n -> p t n"), in_=out_sb
                )

    nc.compile()
    return nc


def _get_program():
    if "nc" not in _CACHE:
        _CACHE["nc"] = _build_program()
    return _CACHE["nc"]


def _prep_inputs(q_data, m_data, bias, nonbatched_bias, query_w, key_w, value_w,
                 output_w):
    """Host-side layout prep -> per-core input maps."""
    scale = KD ** -0.5
    q_data = np.asarray(q_data, np.float32)
    m_data = np.asarray(m_data, np.float32)
    qT = np.ascontiguousarray(q_data.transpose(0, 2, 1)).astype(np.float16)
    mT = np.ascontiguousarray(m_data.transpose(0, 2, 1)).astype(np.float16)
    eb = np.exp(np.asarray(bias, np.float32).reshape(B, KL)).astype(np.float32)
    # device layout [p, t, h, q] with k = t*128 + p
    nbT_f32 = np.ascontiguousarray(
        np.asarray(nonbatched_bias, np.float32)
        .transpose(0, 2, 1)
        .reshape(H, 3, 128, Q)
        .transpose(2, 1, 0, 3)
    )
    nbT = nbT_f32.astype(np.float16)
    enb = np.exp(nbT_f32[:, :, OFF_HEADS, :]).astype(np.float16)
    wq = (np.asarray(query_w, np.float32).reshape(A_DIM, HC) * scale).reshape(
        2, 128, HC
    ).astype(np.float16)
    wk = np.asarray(key_w, np.float32).reshape(A_DIM, HC).reshape(2, 128, HC).astype(
        np.float16
    )
    wv = np.asarray(value_w, np.float32).reshape(A_DIM, HC).reshape(2, 128, HC).astype(
        np.float16
    )
    wo = np.asarray(output_w, np.float32).reshape(HC, OUT).reshape(2, 128, OUT).astype(
        np.float16
    )

    in_maps = []
    for c in range(NCORES):
        s = slice(c * BPC, (c + 1) * BPC)
        in_maps.append(
            {
                "qT": qT[s],
                "mT": mT[s],
                "EB": eb[s],
                "nbT": nbT,
                "ENB": enb,
                "Wq": wq,
                "Wk": wk,
                "Wv": wv,
                "Wo": wo,
            }
        )
    return in_maps


def run_on_cores(in_maps, trace=False, **kwargs):
    nc = _get_program()
    return run_bass_kernel_spmd(
        nc, in_maps, core_ids=list(range(NCORES)), trace=trace, **kwargs
    )


def kernel(q_data, m_data, bias, nonbatched_bias, query_w, key_w, value_w,
           output_w, output_b):
    in_maps = _prep_inputs(
        q_data, m_data, bias, nonbatched_bias, query_w, key_w, value_w, output_w
    )
    res = run_on_cores(in_maps, trace=False)
    out = np.concatenate(
        [r["out"].reshape(BPC, Q, OUT) for r in res.results], axis=0
    ).astype(np.float32)
    out += np.asarray(output_b, np.float32)[None, None, :]
    return out


# revision 30
# speedup vs baseline: 1.0284x; 1.0284x over previous
"""AlphaFold-style NoGatingAttention on 8 Trainium2 NeuronCores.

Problem (hardcoded): B=128, Q=K=384, A=M=256, H=8, KD=VD=32, OUT=256, fp32 I/O.

Strategy: data-parallel over batch (16 per core). Per batch, on-device:
  qT = Wq^T @ q_data^T            [hc, q]   (scale folded into Wq on host)
  kT = Wk^T @ m_data^T            [hc, k]
  v  = m_data^T.T @ Wv            [k, hc]   (natural layout)
  logits^T[k,q] per head = nb^T (PE-injected) + kT_h^T-slices @ qT_h
  E = exp(logits^T)                          (ScalarE, fp16 out)
  wa[q, (h,33)] = E_slice^T @ v_ext          (v_ext has EB=exp(bias) folded in;
                                              col 32 of each head = softmax denom)
  wa_n = wa * recip(denom)  -> PE transpose -> [hc, q]
  out[q, o] = wa_n^T-chunks @ Wo             (+ output_b added on host)

All matmuls fp16 (1 cycle/row on PE); PSUM accumulation fp32.
"""

import numpy as np

import concourse.bass as bass
import concourse.mybir as mybir
import concourse.tile as tile
from concourse import bacc
from concourse.bass_utils import run_bass_kernel_spmd
from concourse.masks import make_identity

B, Q, KL, A_DIM, H, KD, VD, OUT = 128, 384, 384, 256, 8, 32, 32, 256
NCORES = 8
BPC = B // NCORES  # 16 batches per core
HC = H * KD  # 256
F16 = mybir.dt.float16
F32 = mybir.dt.float32

# (t, g) head-pair groups whose nonbatched-bias add runs as a post-exp
# multiply by exp(nb) on DVE / GpSimd instead of a PE PSUM-inject
# (PE/DVE/GpSimd balance).
OFFLOAD_DVE = {
    (0, 3), (1, 3), (2, 3),
    (0, 2), (1, 2), (2, 2),
    (0, 1), (1, 1), (2, 1),
}
OFFLOAD_GPS = set()
OFFLOAD = OFFLOAD_DVE | OFFLOAD_GPS
OFF_HEADS = sorted({g * 2 + j for (_, g) in OFFLOAD for j in range(2)})

_CACHE = {}


def _build_program():
    """Build the per-core Bass/Tile program (identical on all 8 cores)."""
    nc = bacc.Bacc("TRN2", target_bir_lowering=False, debug=False)

    # --- per-core DRAM I/O ---
    qT_d = nc.dram_tensor("qT", [BPC, A_DIM, Q], F16, kind="ExternalInput")
    mT_d = nc.dram_tensor("mT", [BPC, A_DIM, KL], F16, kind="ExternalInput")
    eb_d = nc.dram_tensor("EB", [BPC, KL], F32, kind="ExternalInput")
    nb_d = nc.dram_tensor("nbT", [128, 3, H, Q], F16, kind="ExternalInput")
    enb_d = nc.dram_tensor(
        "ENB", [128, 3, len(OFF_HEADS), Q], F16, kind="ExternalInput"
    )
    wq_d = nc.dram_tensor("Wq", [2, 128, HC], F16, kind="ExternalInput")
    wk_d = nc.dram_tensor("Wk", [2, 128, HC], F16, kind="ExternalInput")
    wv_d = nc.dram_tensor("Wv", [2, 128, HC], F16, kind="ExternalInput")
    wo_d = nc.dram_tensor("Wo", [2, 128, OUT], F16, kind="ExternalInput")
    out_d = nc.dram_tensor("out", [BPC, 3, 128, OUT], F32, kind="ExternalOutput")

    MUL = mybir.AluOpType.mult

    with tile.TileContext(nc) as tc:
        with (
            tc.tile_pool(name="const", bufs=1) as constp,
            tc.tile_pool(name="io", bufs=4) as iop,
            tc.tile_pool(name="work", bufs=3) as workp,
            tc.tile_pool(name="epool", bufs=2) as ep,
            tc.tile_pool(name="psL", bufs=2, space="PSUM") as psL,
            tc.tile_pool(name="psP", bufs=2, space="PSUM") as psP,
            tc.tile_pool(name="psW", bufs=2, space="PSUM") as psW,
        ):
            # --- constants (loaded once) ---
            ident = constp.tile([128, 128], F16)
            make_identity(nc, ident)

            # constants go on the gpsimd (SWDGE) queue so the first batch's
            # input DMAs on the sync queue aren't stuck behind them
            nb_sb = constp.tile([128, 3, H, Q], F16)
            nc.gpsimd.dma_start(out=nb_sb, in_=nb_d[:])
            enb_sb = constp.tile([128, 3, len(OFF_HEADS), Q], F16)
            nc.gpsimd.dma_start(out=enb_sb, in_=enb_d[:])
            off_idx = {h: i for i, h in enumerate(OFF_HEADS)}

            w_sb = {}
            for name, d in (("q", wq_d), ("k", wk_d), ("v", wv_d), ("o", wo_d)):
                w = constp.tile([128, 2, HC], F16, tag=f"w{name}")
                nc.gpsimd.dma_start(out=w, in_=d.rearrange("c p n -> p c n"))
                w_sb[name] = w

            for b in range(BPC):
                # --- input DMAs ---
                qTt = iop.tile([128, 2, Q], F16, tag="qT")
                nc.sync.dma_start(
                    out=qTt, in_=qT_d[b].rearrange("(c p) q -> p c q", p=128)
                )
                mTt = iop.tile([128, 2, KL], F16, tag="mT")
                nc.sync.dma_start(
                    out=mTt, in_=mT_d[b].rearrange("(c p) q -> p c q", p=128)
                )
                ebt = iop.tile([128, 3], F32, tag="eb")
                nc.sync.dma_start(
                    out=ebt, in_=eb_d[b].rearrange("(t p) -> p t", p=128)
                )

                # --- projections: qT [hc, q], kT [hc, k] ---
                proj = {}
                for name, src in (("q", qTt), ("k", mTt)):
                    dst = workp.tile([128, 2, Q], F16, tag=f"p{name}")
                    for t in range(2):
                        ps = psP.tile([128, Q], F32, tag="ps")
                        for c in range(2):
                            nc.tensor.matmul(
                                ps,
                                w_sb[name][:, c, t * 128 : (t + 1) * 128],
                                src[:, c, :],
                                start=(c == 0),
                                stop=(c == 1),
                            )
                        nc.vector.tensor_copy(dst[:, t, :], ps)
                    proj[name] = dst

                # --- v natural [k, hc] scaled by EB, with EB ones-columns ---
                # layout: [128, kt, h*33 + c]; col 33h+32 = EB (denominator trick)
                v_sb = workp.tile([128, 3, H * 33], F16, tag="v")
                for t in range(3):
                    ps = psP.tile([128, HC], F32, tag="ps")
                    for c in range(2):
                        nc.tensor.matmul(
                            ps,
                            mTt[:, c, t * 128 : (t + 1) * 128],
                            w_sb["v"][:, c, :],
                            start=(c == 0),
                            stop=(c == 1),
                        )
                    nc.vector.tensor_scalar(
                        out=v_sb[:, t].rearrange("p (h x) -> p h x", x=33)[:, :, 0:32],
                        in0=ps.rearrange("p (h x) -> p h x", x=32),
                        scalar1=ebt[:, t : t + 1],
                        scalar2=None,
                        op0=MUL,
                    )
                    # ones-columns = EB broadcast into col 32 of each head block
                    nc.vector.tensor_copy(
                        v_sb[:, t].rearrange("p (h x) -> p h x", x=33)[:, :, 32],
                        ebt[:, t : t + 1].broadcast_to((128, H)),
                    )

                # --- logits^T + exp, per k-tile and head-pair ---
                E_sb = ep.tile([128, 3, H, Q], F16, tag="E")
                for t in range(3):
                    for g in range(4):  # head pairs
                        psl = psL.tile([128, 2, 512], F32, tag="psl")
                        off = (t, g) in OFFLOAD
                        if not off:
                            # inject nonbatched bias into PSUM
                            for j in range(2):
                                h = g * 2 + j
                                nc.tensor.matmul(
                                    psl[:, j, 0:Q],
                                    ident,
                                    nb_sb[:, t, h, :],
                                    start=True,
                                    stop=False,
                                )
                        # logits^T[k, q] = kT_h-slice.T @ qT_h; the two heads
                        # use distinct PE row-groups -> run concurrently
                        for j in range(2):
                            h = g * 2 + j
                            r, ch = (h % 4) * 32, h // 4
                            nc.tensor.matmul(
                                psl[:, j, 0:Q],
                                proj["k"][r : r + 32, ch, t * 128 : (t + 1) * 128],
                                proj["q"][r : r + 32, ch, :],
                                start=off,
                                stop=True,
                                tile_position=(r, 0),
                            )
                        nc.scalar.activation(
                            out=E_sb[:, t, g * 2 : g * 2 + 2, :],
                            in_=psl[:, :, 0:Q],
                            func=mybir.ActivationFunctionType.Exp,
                        )
                        if off:
                            oi = off_idx[g * 2]
                            eng = (
                                nc.gpsimd
                                if (t, g) in OFFLOAD_GPS
                                else nc.vector
                            )
                            eng.tensor_tensor(
                                out=E_sb[:, t, g * 2 : g * 2 + 2, :],
                                in0=E_sb[:, t, g * 2 : g * 2 + 2, :],
                                in1=enb_sb[:, t, oi : oi + 2, :],
                                op=MUL,
                            )

                # --- weighted average + denominators ---
                waT = workp.tile([128, 2, Q], F16, tag="waT")
                for qb in range(3):
                    psw = psW.tile([128, H * 33], F32, tag="ps")
                    for h in range(8):
                        for t in range(3):
                            nc.tensor.matmul(
                                psw[:, h * 33 : h * 33 + 33],
                                E_sb[:, t, h, qb * 128 : (qb + 1) * 128],
                                v_sb[:, t, h * 33 : h * 33 + 33],
                                start=(t == 0),
                                stop=(t == 2),
                            )
                    rden = workp.tile([128, H], F32, tag="rden")
                    nc.vector.reciprocal(
                        rden, psw.rearrange("p (h x) -> p h x", x=33)[:, :, 32]
                    )
                    # normalize all heads in one op: in1 is a stride-0
                    # broadcast of each head's recip denom across its 32 cols
                    wa_n = workp.tile([128, HC], F16, tag="wan")
                    nc.vector.tensor_tensor(
                        out=wa_n.rearrange("p (h x) -> p h x", x=32),
                        in0=psw.rearrange("p (h x) -> p h x", x=33)[:, :, 0:32],
                        in1=rden.rearrange("p (h o) -> p h o", o=1).broadcast_to(
                            (128, H, 32)
                        ),
                        op=MUL,
                    )
                    # transpose to [hc, q]
                    for c in range(2):
                        pst = psW.tile([128, 128], F16, tag="ps")
                        nc.tensor.transpose(
                            pst, wa_n[:, c * 128 : (c + 1) * 128], ident
                        )
                        nc.vector.tensor_copy(
                            waT[:, c, qb * 128 : (qb + 1) * 128], pst
                        )

                # --- output projection ---
                out_sb = iop.tile([128, 3, OUT], F32, tag="out")
                for qb in range(3):
                    pso = psW.tile([128, OUT], F32, tag="ps")
                    for c in range(2):
                        nc.tensor.matmul(
                            pso,
                            waT[:, c, qb * 128 : (qb + 1) * 128],
                            w_sb["o"][:, c, :],
                            start=(c == 0),
                            stop=(c == 1),
                        )
                    nc.vector.tensor_copy(out_sb[:, qb, :], pso)

                nc.sync.dma_start(
                    out=out_d[b].rearrange("t p n -> p t n"), in_=out_sb
                )

    nc.compile()
    return nc


def _get_program():
    if "nc" not in _CACHE:
        _CACHE["nc"] = _build_program()
    return _CACHE["nc"]


def _prep_inputs(q_data, m_data, bias, nonbatched_bias, query_w, key_w, value_w,
                 output_w):
    """Host-side layout prep -> per-core input maps."""
    scale = KD ** -0.5
    q_data = np.asarray(q_data, np.float32)
    m_data = np.asarray(m_data, np.float32)
    qT = np.ascontiguousarray(q_data.transpose(0, 2, 1)).astype(np.float16)
    mT = np.ascontiguousarray(m_data.transpose(0, 2, 1)).astype(np.float16)
    eb = np.exp(np.asarray(bias, np.float32).reshape(B, KL)).astype(np.float32)
    # device layout [p, t, h, q] with k = t*128 + p
    nbT_f32 = np.ascontiguousarray(
        np.asarray(nonbatched_bias, np.float32)
        .transpose(0, 2, 1)
        .reshape(H, 3, 128, Q)
        .transpose(2, 1, 0, 3)
    )
    nbT = nbT_f32.astype(np.float16)
    enb = np.exp(nbT_f32[:, :, OFF_HEADS, :]).astype(np.float16)
    wq = (np.asarray(query_w, np.float32).reshape(A_DIM, HC) * scale).reshape(
        2, 128, HC
    ).astype(np.float16)
    wk = np.asarray(key_w, np.float32).reshape(A_DIM, HC).reshape(2, 128, HC).astype(
        np.float16
    )
    wv = np.asarray(value_w, np.float32).reshape(A_DIM, HC).reshape(2, 128, HC).astype(
        np.float16
    )
    wo = np.asarray(output_w, np.float32).reshape(HC, OUT).reshape(2, 128, OUT).astype(
        np.float16
    )

    in_maps = []
    for c in range(NCORES):
        s = slice(c * BPC, (c + 1) * BPC)
        in_maps.append(
            {
                "qT": qT[s],
                "mT": mT[s],
                "EB": eb[s],
                "nbT": nbT,
                "ENB": enb,
                "Wq": wq,
                "Wk": wk,
                "Wv": wv,
                "Wo": wo,
            }
        )
    return in_maps


def run_on_cores(in_maps, trace=False, **kwargs):
    nc = _get_program()
    return run_bass_kernel_spmd(
        nc, in_maps, core_ids=list(range(NCORES)), trace=trace, **kwargs
    )


def kernel(q_data, m_data, bias, nonbatched_bias, query_w, key_w, value_w,
           output_w, output_b):
    in_maps = _prep_inputs(
        q_data, m_data, bias, nonbatched_bias, query_w, key_w, value_w, output_w
    )
    res = run_on_cores(in_maps, trace=False)
    out = np.concatenate(
        [r["out"].reshape(BPC, Q, OUT) for r in res.results], axis=0
    ).astype(np.float32)
    out += np.asarray(output_b, np.float32)[None, None, :]
    return out


# revision 37
# speedup vs baseline: 1.0579x; 1.0286x over previous
"""AlphaFold-style NoGatingAttention on 8 Trainium2 NeuronCores.

Problem (hardcoded): B=128, Q=K=384, A=M=256, H=8, KD=VD=32, OUT=256, fp32 I/O.

Strategy: data-parallel over batch (16 per core). Per batch, on-device:
  qT = Wq^T @ q_data^T            [hc, q]   (scale folded into Wq on host)
  kT = Wk^T @ m_data^T            [hc, k]
  v  = m_data^T.T @ Wv            [k, hc]   (natural layout)
  logits^T[k,q] per head = nb^T (PE-injected) + kT_h^T-slices @ qT_h
  E = exp(logits^T)                          (ScalarE, fp16 out)
  wa[q, (h,33)] = E_slice^T @ v_ext          (v_ext has EB=exp(bias) folded in;
                                              col 32 of each head = softmax denom)
  wa_n = wa * recip(denom)  -> PE transpose -> [hc, q]
  out[q, o] = wa_n^T-chunks @ Wo             (+ output_b added on host)

All matmuls fp16 (1 cycle/row on PE); PSUM accumulation fp32.
"""

import numpy as np

import concourse.bass as bass
import concourse.mybir as mybir
import concourse.tile as tile
from concourse import bacc
from concourse.bass_utils import run_bass_kernel_spmd
from concourse.masks import make_identity

B, Q, KL, A_DIM, H, KD, VD, OUT = 128, 384, 384, 256, 8, 32, 32, 256
NCORES = 8
BPC = B // NCORES  # 16 batches per core
HC = H * KD  # 256
F16 = mybir.dt.float16
F32 = mybir.dt.float32

# (t, g) head-pair groups whose nonbatched-bias add runs as a post-exp
# multiply by exp(nb) on DVE / GpSimd instead of a PE PSUM-inject
# (PE/DVE/GpSimd balance).
OFFLOAD_DVE = {
    (0, 3), (1, 3), (2, 3),
    (0, 2), (1, 2), (2, 2),
    (0, 1), (1, 1), (2, 1),
}
OFFLOAD_GPS = set()
OFFLOAD = OFFLOAD_DVE | OFFLOAD_GPS
OFF_HEADS = sorted({g * 2 + j for (_, g) in OFFLOAD for j in range(2)})

_CACHE = {}


def _build_program():
    """Build the per-core Bass/Tile program (identical on all 8 cores)."""
    nc = bacc.Bacc("TRN2", target_bir_lowering=False, debug=False)

    # --- per-core DRAM I/O ---
    qT_d = nc.dram_tensor("qT", [BPC, A_DIM, Q], F16, kind="ExternalInput")
    mT_d = nc.dram_tensor("mT", [BPC, A_DIM, KL], F16, kind="ExternalInput")
    eba_d = nc.dram_tensor("EB", [128, BPC, 3], F32, kind="ExternalInput")
    nb_d = nc.dram_tensor("nbT", [128, 3, H, Q], F16, kind="ExternalInput")
    enb_d = nc.dram_tensor(
        "ENB", [128, 3, len(OFF_HEADS), Q], F16, kind="ExternalInput"
    )
    wq_d = nc.dram_tensor("Wq", [2, 128, HC], F16, kind="ExternalInput")
    wk_d = nc.dram_tensor("Wk", [2, 128, HC], F16, kind="ExternalInput")
    wv_d = nc.dram_tensor("Wv", [2, 128, HC], F16, kind="ExternalInput")
    wo_d = nc.dram_tensor("Wo", [2, 128, OUT], F16, kind="ExternalInput")
    out_d = nc.dram_tensor("out", [BPC, 3, 128, OUT], F32, kind="ExternalOutput")

    MUL = mybir.AluOpType.mult

    with tile.TileContext(nc) as tc:
        with (
            tc.tile_pool(name="const", bufs=1) as constp,
            tc.tile_pool(name="io", bufs=4) as iop,
            tc.tile_pool(name="work", bufs=3) as workp,
            tc.tile_pool(name="epool", bufs=2) as ep,
            tc.tile_pool(name="psL", bufs=2, space="PSUM") as psL,
            tc.tile_pool(name="psP", bufs=2, space="PSUM") as psP,
            tc.tile_pool(name="psW", bufs=2, space="PSUM") as psW,
        ):
            # --- constants (loaded once) ---
            ident = constp.tile([128, 128], F16)
            make_identity(nc, ident)

            # constants go on the gpsimd (SWDGE) queue so the first batch's
            # input DMAs on the sync queue aren't stuck behind them; small
            # weights first so projections can start immediately
            w_sb = {}
            for name, d in (("q", wq_d), ("k", wk_d), ("v", wv_d), ("o", wo_d)):
                w = constp.tile([128, 2, HC], F16, tag=f"w{name}")
                nc.gpsimd.dma_start(out=w, in_=d.rearrange("c p n -> p c n"))
                w_sb[name] = w

            eba = constp.tile([128, BPC, 3], F32)
            nc.gpsimd.dma_start(out=eba, in_=eba_d[:])

            nb_sb = constp.tile([128, 3, H, Q], F16)
            nc.gpsimd.dma_start(out=nb_sb, in_=nb_d[:])
            enb_sb = constp.tile([128, 3, len(OFF_HEADS), Q], F16)
            nc.gpsimd.dma_start(out=enb_sb, in_=enb_d[:])
            off_idx = {h: i for i, h in enumerate(OFF_HEADS)}

            for b in range(BPC):
                # --- input DMAs ---
                qTt = iop.tile([128, 2, Q], F16, tag="qT")
                nc.sync.dma_start(
                    out=qTt, in_=qT_d[b].rearrange("(c p) q -> p c q", p=128)
                )
                mTt = iop.tile([128, 2, KL], F16, tag="mT")
                nc.sync.dma_start(
                    out=mTt, in_=mT_d[b].rearrange("(c p) q -> p c q", p=128)
                )
                ebt = eba[:, b, :]

                # --- projections: qT [hc, q], kT [hc, k] ---
                proj = {}
                for name, src in (("q", qTt), ("k", mTt)):
                    dst = workp.tile([128, 2, Q], F16, tag=f"p{name}")
                    for t in range(2):
                        ps = psP.tile([128, Q], F32, tag="ps")
                        for c in range(2):
                            nc.tensor.matmul(
                                ps,
                                w_sb[name][:, c, t * 128 : (t + 1) * 128],
                                src[:, c, :],
                                start=(c == 0),
                                stop=(c == 1),
                            )
                        nc.vector.tensor_copy(dst[:, t, :], ps)
                    proj[name] = dst

                # --- v natural [k, hc] scaled by EB, with EB ones-columns ---
                # layout: [128, kt, h*33 + c]; col 33h+32 = EB (denominator trick)
                v_sb = workp.tile([128, 3, H * 33], F16, tag="v")
                for t in range(3):
                    ps = psP.tile([128, HC], F32, tag="ps")
                    for c in range(2):
                        nc.tensor.matmul(
                            ps,
                            mTt[:, c, t * 128 : (t + 1) * 128],
                            w_sb["v"][:, c, :],
                            start=(c == 0),
                            stop=(c == 1),
                        )
                    nc.vector.tensor_scalar(
                        out=v_sb[:, t].rearrange("p (h x) -> p h x", x=33)[:, :, 0:32],
                        in0=ps.rearrange("p (h x) -> p h x", x=32),
                        scalar1=ebt[:, t : t + 1],
                        scalar2=None,
                        op0=MUL,
                    )
                    # ones-columns = EB broadcast into col 32 of each head block
                    nc.vector.tensor_copy(
                        v_sb[:, t].rearrange("p (h x) -> p h x", x=33)[:, :, 32],
                        ebt[:, t : t + 1].broadcast_to((128, H)),
                    )

                # --- logits^T + exp, per k-tile and head-pair ---
                E_sb = ep.tile([128, 3, H, Q], F16, tag="E")
                for t in range(3):
                    for g in range(4):  # head pairs
                        psl = psL.tile([128, 2, 512], F32, tag="psl")
                        off = (t, g) in OFFLOAD
                        if not off:
                            # inject nonbatched bias into PSUM
                            for j in range(2):
                                h = g * 2 + j
                                nc.tensor.matmul(
                                    psl[:, j, 0:Q],
                                    ident,
                                    nb_sb[:, t, h, :],
                                    start=True,
                                    stop=False,
                                )
                        # logits^T[k, q] = kT_h-slice.T @ qT_h; the two heads
                        # use distinct PE row-groups -> run concurrently
                        for j in range(2):
                            h = g * 2 + j
                            r, ch = (h % 4) * 32, h // 4
                            nc.tensor.matmul(
                                psl[:, j, 0:Q],
                                proj["k"][r : r + 32, ch, t * 128 : (t + 1) * 128],
                                proj["q"][r : r + 32, ch, :],
                                start=off,
                                stop=True,
                                tile_position=(r, 0),
                            )
                        nc.scalar.activation(
                            out=E_sb[:, t, g * 2 : g * 2 + 2, :],
                            in_=psl[:, :, 0:Q],
                            func=mybir.ActivationFunctionType.Exp,
                        )
                        if off:
                            oi = off_idx[g * 2]
                            eng = (
                                nc.gpsimd
                                if (t, g) in OFFLOAD_GPS
                                else nc.vector
                            )
                            eng.tensor_tensor(
                                out=E_sb[:, t, g * 2 : g * 2 + 2, :],
                                in0=E_sb[:, t, g * 2 : g * 2 + 2, :],
                                in1=enb_sb[:, t, oi : oi + 2, :],
                                op=MUL,
                            )

                # --- weighted average + denominators ---
                waT = workp.tile([128, 2, Q], F16, tag="waT")
                for qb in range(3):
                    psw = psW.tile([128, H * 33], F32, tag="ps")
                    for h in range(8):
                        for t in range(3):
                            nc.tensor.matmul(
                                psw[:, h * 33 : h * 33 + 33],
                                E_sb[:, t, h, qb * 128 : (qb + 1) * 128],
                                v_sb[:, t, h * 33 : h * 33 + 33],
                                start=(t == 0),
                                stop=(t == 2),
                            )
                    rden = workp.tile([128, H], F32, tag="rden")
                    nc.vector.reciprocal(
                        rden, psw.rearrange("p (h x) -> p h x", x=33)[:, :, 32]
                    )
                    # normalize all heads in one op: in1 is a stride-0
                    # broadcast of each head's recip denom across its 32 cols
                    wa_n = workp.tile([128, HC], F16, tag="wan")
                    nc.vector.tensor_tensor(
                        out=wa_n.rearrange("p (h x) -> p h x", x=32),
                        in0=psw.rearrange("p (h x) -> p h x", x=33)[:, :, 0:32],
                        in1=rden.rearrange("p (h o) -> p h o", o=1).broadcast_to(
                            (128, H, 32)
                        ),
                        op=MUL,
                    )
                    # transpose to [hc, q]; both 128x128 transposes land in one
                    # PSUM tile so a single DVE op evacuates them
                    pst = psW.tile([128, 2, 128], F16, tag="ps")
                    for c in range(2):
                        nc.tensor.transpose(
                            pst[:, c, :], wa_n[:, c * 128 : (c + 1) * 128], ident
                        )
                    nc.vector.tensor_copy(
                        waT[:, :, qb * 128 : (qb + 1) * 128], pst
                    )

                # --- output projection ---
                out_sb = iop.tile([128, 3, OUT], F32, tag="out")
                for qb in range(3):
                    pso = psW.tile([128, OUT], F32, tag="ps")
                    for c in range(2):
                        nc.tensor.matmul(
                            pso,
                            waT[:, c, qb * 128 : (qb + 1) * 128],
                            w_sb["o"][:, c, :],
                            start=(c == 0),
                            stop=(c == 1),
                        )
                    nc.scalar.copy(out_sb[:, qb, :], pso)

                nc.sync.dma_start(
                    out=out_d[b].rearrange("t p n -> p t n"), in_=out_sb
                )

    nc.compile()
    return nc


def _get_program():
    if "nc" not in _CACHE:
        _CACHE["nc"] = _build_program()
    return _CACHE["nc"]


def _prep_inputs(q_data, m_data, bias, nonbatched_bias, query_w, key_w, value_w,
                 output_w):
    """Host-side layout prep -> per-core input maps."""
    scale = KD ** -0.5
    q_data = np.asarray(q_data, np.float32)
    m_data = np.asarray(m_data, np.float32)
    qT = np.ascontiguousarray(q_data.transpose(0, 2, 1)).astype(np.float16)
    mT = np.ascontiguousarray(m_data.transpose(0, 2, 1)).astype(np.float16)
    # [128, BPC, 3] per core with k = t*128 + p
    eb = np.ascontiguousarray(
        np.exp(np.asarray(bias, np.float32).reshape(B, KL))
        .reshape(NCORES, BPC, 3, 128)
        .transpose(0, 3, 1, 2)
    ).astype(np.float32)
    # device layout [p, t, h, q] with k = t*128 + p
    nbT_f32 = np.ascontiguousarray(
        np.asarray(nonbatched_bias, np.float32)
        .transpose(0, 2, 1)
        .reshape(H, 3, 128, Q)
        .transpose(2, 1, 0, 3)
    )
    nbT = nbT_f32.astype(np.float16)
    enb = np.exp(nbT_f32[:, :, OFF_HEADS, :]).astype(np.float16)
    wq = (np.asarray(query_w, np.float32).reshape(A_DIM, HC) * scale).reshape(
        2, 128, HC
    ).astype(np.float16)
    wk = np.asarray(key_w, np.float32).reshape(A_DIM, HC).reshape(2, 128, HC).astype(
        np.float16
    )
    wv = np.asarray(value_w, np.float32).reshape(A_DIM, HC).reshape(2, 128, HC).astype(
        np.float16
    )
    wo = np.asarray(output_w, np.float32).reshape(HC, OUT).reshape(2, 128, OUT).astype(
        np.float16
    )

    in_maps = []
    for c in range(NCORES):
        s = slice(c * BPC, (c + 1) * BPC)
        in_maps.append(
            {
                "qT": qT[s],
                "mT": mT[s],
                "EB": eb[c],
                "nbT": nbT,
                "ENB": enb,
                "Wq": wq,
                "Wk": wk,
                "Wv": wv,
                "Wo": wo,
            }
        )
    return in_maps


def run_on_cores(in_maps, trace=False, **kwargs):
    nc = _get_program()
    return run_bass_kernel_spmd(
        nc, in_maps, core_ids=list(range(NCORES)), trace=trace, **kwargs
    )


def kernel(q_data, m_data, bias, nonbatched_bias, query_w, key_w, value_w,
           output_w, output_b):
    in_maps = _prep_inputs(
        q_data, m_data, bias, nonbatched_bias, query_w, key_w, value_w, output_w
    )
    res = run_on_cores(in_maps, trace=False)
    out = np.concatenate(
        [r["out"].reshape(BPC, Q, OUT) for r in res.results], axis=0
    ).astype(np.float32)
    out += np.asarray(output_b, np.float32)[None, None, :]
    return out


# revision 38
# speedup vs baseline: 1.0762x; 1.0173x over previous
"""AlphaFold-style NoGatingAttention on 8 Trainium2 NeuronCores.

Problem (hardcoded): B=128, Q=K=384, A=M=256, H=8, KD=VD=32, OUT=256, fp32 I/O.

Strategy: data-parallel over batch (16 per core). Per batch, on-device:
  qT = Wq^T @ q_data^T            [hc, q]   (scale folded into Wq on host)
  kT = Wk^T @ m_data^T            [hc, k]
  v  = m_data^T.T @ Wv            [k, hc]   (natural layout)
  logits^T[k,q] per head = nb^T (PE-injected) + kT_h^T-slices @ qT_h
  E = exp(logits^T)                          (ScalarE, fp16 out)
  wa[q, (h,33)] = E_slice^T @ v_ext          (v_ext has EB=exp(bias) folded in;
                                              col 32 of each head = softmax denom)
  wa_n = wa * recip(denom)  -> PE transpose -> [hc, q]
  out[q, o] = wa_n^T-chunks @ Wo             (+ output_b added on host)

All matmuls fp16 (1 cycle/row on PE); PSUM accumulation fp32.
"""

import numpy as np

import concourse.bass as bass
import concourse.mybir as mybir
import concourse.tile as tile
from concourse import bacc
from concourse.bass_utils import run_bass_kernel_spmd
from concourse.masks import make_identity

B, Q, KL, A_DIM, H, KD, VD, OUT = 128, 384, 384, 256, 8, 32, 32, 256
NCORES = 8
BPC = B // NCORES  # 16 batches per core
HC = H * KD  # 256
F16 = mybir.dt.float16
F32 = mybir.dt.float32

# (t, g) head-pair groups whose nonbatched-bias add runs as a post-exp
# multiply by exp(nb) on DVE / GpSimd instead of a PE PSUM-inject
# (PE/DVE/GpSimd balance).
OFFLOAD_DVE = {
    (0, 3), (1, 3), (2, 3),
    (0, 2), (1, 2), (2, 2),
    (0, 1), (1, 1), (2, 1),
}
OFFLOAD_GPS = set()
OFFLOAD = OFFLOAD_DVE | OFFLOAD_GPS
OFF_HEADS = sorted({g * 2 + j for (_, g) in OFFLOAD for j in range(2)})

_CACHE = {}


def _build_program():
    """Build the per-core Bass/Tile program (identical on all 8 cores)."""
    nc = bacc.Bacc("TRN2", target_bir_lowering=False, debug=False)

    # --- per-core DRAM I/O ---
    qT_d = nc.dram_tensor("qT", [BPC, A_DIM, Q], F16, kind="ExternalInput")
    mT_d = nc.dram_tensor("mT", [BPC, A_DIM, KL], F16, kind="ExternalInput")
    eba_d = nc.dram_tensor("EB", [128, BPC, 3], F32, kind="ExternalInput")
    nb_d = nc.dram_tensor("nbT", [128, 3, H, Q], F16, kind="ExternalInput")
    enb_d = nc.dram_tensor(
        "ENB", [128, 3, len(OFF_HEADS), Q], F16, kind="ExternalInput"
    )
    wq_d = nc.dram_tensor("Wq", [2, 128, HC], F16, kind="ExternalInput")
    wk_d = nc.dram_tensor("Wk", [2, 128, HC], F16, kind="ExternalInput")
    wv_d = nc.dram_tensor("Wv", [2, 128, HC], F16, kind="ExternalInput")
    wo_d = nc.dram_tensor("Wo", [2, 128, OUT], F16, kind="ExternalInput")
    out_d = nc.dram_tensor("out", [BPC, 3, 128, OUT], F32, kind="ExternalOutput")

    MUL = mybir.AluOpType.mult

    with tile.TileContext(nc) as tc:
        with (
            tc.tile_pool(name="const", bufs=1) as constp,
            tc.tile_pool(name="io", bufs=4) as iop,
            tc.tile_pool(name="work", bufs=3) as workp,
            tc.tile_pool(name="epool", bufs=2) as ep,
            tc.tile_pool(name="psL", bufs=2, space="PSUM") as psL,
            tc.tile_pool(name="psP", bufs=2, space="PSUM") as psP,
            tc.tile_pool(name="psW", bufs=2, space="PSUM") as psW,
        ):
            # --- constants (loaded once) ---
            ident = constp.tile([128, 128], F16)
            make_identity(nc, ident)

            # constants go on the gpsimd (SWDGE) queue so the first batch's
            # input DMAs on the sync queue aren't stuck behind them; small
            # weights first so projections can start immediately
            w_sb = {}
            for name, d in (("q", wq_d), ("k", wk_d), ("v", wv_d), ("o", wo_d)):
                w = constp.tile([128, 2, HC], F16, tag=f"w{name}")
                nc.gpsimd.dma_start(out=w, in_=d.rearrange("c p n -> p c n"))
                w_sb[name] = w

            eba = constp.tile([128, BPC, 3], F32)
            nc.gpsimd.dma_start(out=eba, in_=eba_d[:])

            nb_sb = constp.tile([128, 3, H, Q], F16)
            nc.gpsimd.dma_start(out=nb_sb, in_=nb_d[:])
            enb_sb = constp.tile([128, 3, len(OFF_HEADS), Q], F16)
            nc.gpsimd.dma_start(out=enb_sb, in_=enb_d[:])
            off_idx = {h: i for i, h in enumerate(OFF_HEADS)}

            for b in range(BPC):
                # --- input DMAs ---
                qTt = iop.tile([128, 2, Q], F16, tag="qT")
                nc.sync.dma_start(
                    out=qTt, in_=qT_d[b].rearrange("(c p) q -> p c q", p=128)
                )
                mTt = iop.tile([128, 2, KL], F16, tag="mT")
                nc.sync.dma_start(
                    out=mTt, in_=mT_d[b].rearrange("(c p) q -> p c q", p=128)
                )
                ebt = eba[:, b, :]

                # --- projections: qT [hc, q], kT [hc, k] ---
                proj = {}
                for name, src in (("q", qTt), ("k", mTt)):
                    dst = workp.tile([128, 2, Q], F16, tag=f"p{name}")
                    for t in range(2):
                        ps = psP.tile([128, Q], F32, tag="ps")
                        for c in range(2):
                            nc.tensor.matmul(
                                ps,
                                w_sb[name][:, c, t * 128 : (t + 1) * 128],
                                src[:, c, :],
                                start=(c == 0),
                                stop=(c == 1),
                            )
                        nc.vector.tensor_copy(dst[:, t, :], ps)
                    proj[name] = dst

                # --- v natural [k, hc] scaled by EB, with EB ones-columns ---
                # layout: [128, kt, h*33 + c]; col 33h+32 = EB (denominator trick)
                v_sb = workp.tile([128, 3, H * 33], F16, tag="v")
                for t in range(3):
                    ps = psP.tile([128, HC], F32, tag="ps")
                    for c in range(2):
                        nc.tensor.matmul(
                            ps,
                            mTt[:, c, t * 128 : (t + 1) * 128],
                            w_sb["v"][:, c, :],
                            start=(c == 0),
                            stop=(c == 1),
                        )
                    nc.vector.tensor_scalar(
                        out=v_sb[:, t].rearrange("p (h x) -> p h x", x=33)[:, :, 0:32],
                        in0=ps.rearrange("p (h x) -> p h x", x=32),
                        scalar1=ebt[:, t : t + 1],
                        scalar2=None,
                        op0=MUL,
                    )
                    # ones-columns = EB broadcast into col 32 of each head block
                    nc.vector.tensor_copy(
                        v_sb[:, t].rearrange("p (h x) -> p h x", x=33)[:, :, 32],
                        ebt[:, t : t + 1].broadcast_to((128, H)),
                    )

                # --- logits^T + exp, per k-tile and head-pair ---
                E_sb = ep.tile([128, 3, H, Q], F16, tag="E")
                for t in range(3):
                    # two waves of two head-pairs; within a wave all four QK
                    # matmuls hit distinct PE row-groups (0/32/64/96) and are
                    # issued back-to-back -> 4-way concurrency
                    for w in range(2):
                        pls = []
                        for g in (w * 2, w * 2 + 1):
                            psl = psL.tile([128, 2, 512], F32, tag="psl")
                            pls.append(psl)
                            if (t, g) not in OFFLOAD:
                                for j in range(2):
                                    h = g * 2 + j
                                    nc.tensor.matmul(
                                        psl[:, j, 0:Q],
                                        ident,
                                        nb_sb[:, t, h, :],
                                        start=True,
                                        stop=False,
                                    )
                        for gi, g in enumerate((w * 2, w * 2 + 1)):
                            off = (t, g) in OFFLOAD
                            for j in range(2):
                                h = g * 2 + j
                                r, ch = (h % 4) * 32, h // 4
                                nc.tensor.matmul(
                                    pls[gi][:, j, 0:Q],
                                    proj["k"][
                                        r : r + 32, ch, t * 128 : (t + 1) * 128
                                    ],
                                    proj["q"][r : r + 32, ch, :],
                                    start=off,
                                    stop=True,
                                    tile_position=(r, 0),
                                )
                        for gi, g in enumerate((w * 2, w * 2 + 1)):
                            nc.scalar.activation(
                                out=E_sb[:, t, g * 2 : g * 2 + 2, :],
                                in_=pls[gi][:, :, 0:Q],
                                func=mybir.ActivationFunctionType.Exp,
                            )
                            if (t, g) in OFFLOAD:
                                oi = off_idx[g * 2]
                                eng = (
                                    nc.gpsimd
                                    if (t, g) in OFFLOAD_GPS
                                    else nc.vector
                                )
                                eng.tensor_tensor(
                                    out=E_sb[:, t, g * 2 : g * 2 + 2, :],
                                    in0=E_sb[:, t, g * 2 : g * 2 + 2, :],
                                    in1=enb_sb[:, t, oi : oi + 2, :],
                                    op=MUL,
                                )

                # --- weighted average + denominators ---
                waT = workp.tile([128, 2, Q], F16, tag="waT")
                for qb in range(3):
                    psw = psW.tile([128, H * 33], F32, tag="ps")
                    for h in range(8):
                        for t in range(3):
                            nc.tensor.matmul(
                                psw[:, h * 33 : h * 33 + 33],
                                E_sb[:, t, h, qb * 128 : (qb + 1) * 128],
                                v_sb[:, t, h * 33 : h * 33 + 33],
                                start=(t == 0),
                                stop=(t == 2),
                            )
                    rden = workp.tile([128, H], F32, tag="rden")
                    nc.vector.reciprocal(
                        rden, psw.rearrange("p (h x) -> p h x", x=33)[:, :, 32]
                    )
                    # normalize all heads in one op: in1 is a stride-0
                    # broadcast of each head's recip denom across its 32 cols
                    wa_n = workp.tile([128, HC], F16, tag="wan")
                    nc.vector.tensor_tensor(
                        out=wa_n.rearrange("p (h x) -> p h x", x=32),
                        in0=psw.rearrange("p (h x) -> p h x", x=33)[:, :, 0:32],
                        in1=rden.rearrange("p (h o) -> p h o", o=1).broadcast_to(
                            (128, H, 32)
                        ),
                        op=MUL,
                    )
                    # transpose to [hc, q]; both 128x128 transposes land in one
                    # PSUM tile so a single DVE op evacuates them
                    pst = psW.tile([128, 2, 128], F16, tag="ps")
                    for c in range(2):
                        nc.tensor.transpose(
                            pst[:, c, :], wa_n[:, c * 128 : (c + 1) * 128], ident
                        )
                    nc.vector.tensor_copy(
                        waT[:, :, qb * 128 : (qb + 1) * 128], pst
                    )

                # --- output projection ---
                out_sb = iop.tile([128, 3, OUT], F32, tag="out")
                for qb in range(3):
                    pso = psW.tile([128, OUT], F32, tag="ps")
                    for c in range(2):
                        nc.tensor.matmul(
                            pso,
                            waT[:, c, qb * 128 : (qb + 1) * 128],
                            w_sb["o"][:, c, :],
                            start=(c == 0),
                            stop=(c == 1),
                        )
                    nc.scalar.copy(out_sb[:, qb, :], pso)

                nc.sync.dma_start(
                    out=out_d[b].rearrange("t p n -> p t n"), in_=out_sb
                )

    nc.compile()
    return nc


def _get_program():
    if "nc" not in _CACHE:
        _CACHE["nc"] = _build_program()
    return _CACHE["nc"]


def _prep_inputs(q_data, m_data, bias, nonbatched_bias, query_w, key_w, value_w,
                 output_w):
    """Host-side layout prep -> per-core input maps."""
    scale = KD ** -0.5
    q_data = np.asarray(q_data, np.float32)
    m_data = np.asarray(m_data, np.float32)
    qT = np.ascontiguousarray(q_data.transpose(0, 2, 1)).astype(np.float16)
    mT = np.ascontiguousarray(m_data.transpose(0, 2, 1)).astype(np.float16)
    # [128, BPC, 3] per core with k = t*128 + p
    eb = np.ascontiguousarray(
        np.exp(np.asarray(bias, np.float32).reshape(B, KL))
        .reshape(NCORES, BPC, 3, 128)
        .transpose(0, 3, 1, 2)
    ).astype(np.float32)
    # device layout [p, t, h, q] with k = t*128 + p
    nbT_f32 = np.ascontiguousarray(
        np.asarray(nonbatched_bias, np.float32)
        .transpose(0, 2, 1)
        .reshape(H, 3, 128, Q)
        .transpose(2, 1, 0, 3)
    )
    nbT = nbT_f32.astype(np.float16)
    enb = np.exp(nbT_f32[:, :, OFF_HEADS, :]).astype(np.float16)
    wq = (np.asarray(query_w, np.float32).reshape(A_DIM, HC) * scale).reshape(
        2, 128, HC
    ).astype(np.float16)
    wk = np.asarray(key_w, np.float32).reshape(A_DIM, HC).reshape(2, 128, HC).astype(
        np.float16
    )
    wv = np.asarray(value_w, np.float32).reshape(A_DIM, HC).reshape(2, 128, HC).astype(
        np.float16
    )
    wo = np.asarray(output_w, np.float32).reshape(HC, OUT).reshape(2, 128, OUT).astype(
        np.float16
    )

    in_maps = []
    for c in range(NCORES):
        s = slice(c * BPC, (c + 1) * BPC)
        in_maps.append(
            {
                "qT": qT[s],
                "mT": mT[s],
                "EB": eb[c],
                "nbT": nbT,
                "ENB": enb,
                "Wq": wq,
                "Wk": wk,
                "Wv": wv,
                "Wo": wo,
            }
        )
    return in_maps


def run_on_cores(in_maps, trace=False, **kwargs):
    nc = _get_program()
    return run_bass_kernel_spmd(
        nc, in_maps, core_ids=list(range(NCORES)), trace=trace, **kwargs
    )


def kernel(q_data, m_data, bias, nonbatched_bias, query_w, key_w, value_w,
           output_w, output_b):
    in_maps = _prep_inputs(
        q_data, m_data, bias, nonbatched_bias, query_w, key_w, value_w, output_w
    )
    res = run_on_cores(in_maps, trace=False)
    out = np.concatenate(
        [r["out"].reshape(BPC, Q, OUT) for r in res.results], axis=0
    ).astype(np.float32)
    out += np.asarray(output_b, np.float32)[None, None, :]
    return out
